# revision 5
# baseline (speedup 1.0000x reference)
"""ChamferLoss Trainium2 kernel.

Strategy (per core, data-parallel over batch: 16 batches / 8 cores = 2 each):
  pdist[b,i,j] = ||x_i||^2 + ||y_j||^2 - 2 x_i.y_j   (first 3 channels)
  loss = mean_bj(min_i pdist) + mean_bi(min_j pdist)

We compute m = -pdist via a single K=13 bf16 augmented matmul (split-precision
hi/lo trick gives fp32-class accuracy at bf16 PE speed):
  x-side rows: [xh(3), xh(3), xl(3), -rxh, -rxl, -1, -1]
  y-side rows: [Yh(3), Yl(3), Yh(3),  1,    1,  Ryh, Ryl],  Y = 2y, Ry=||y||^2
  sum_k xrow_k * yrow_k = 2x.y - rx - ry = -pdist
Then min -> max: rowmax (DVE reduce over free axis), colmax (tensor_tensor max
accumulation split DVE/GPSIMD), partition reduction via partition_all_reduce.
PSUM tiles are cast to bf16 by the Scalar engine (ACT) so the DVE runs in
2x packed mode; bf16 relative rounding of pdist perturbs the final loss by
only ~1e-5 (verified numerically).

Host-side: shard batches across 8 cores, run SPMD, sum per-core partial sums,
negate, divide by B*N.
"""

from contextlib import ExitStack

import numpy as np

import concourse.bass as bass
import concourse.bacc as bacc
import concourse.tile as tile
from concourse import bass_isa, mybir
from concourse.bass_utils import run_bass_kernel_spmd

F32 = mybir.dt.float32
BF16 = mybir.dt.bfloat16
AX = mybir.AxisListType
OP = mybir.AluOpType

NEG_BIG = -3.0e38  # ~bf16 lowest; colmax accumulator init

# full problem: B=16, N=4096, C=6 (first 3 channels used), 8 cores
B_FULL = 16
N_FULL = 4096
C_FULL = 6
N_CORES = 8


def build_nc(b_loc=2, n=4096, c_in=6, r2_dve_start=16, num_devices=8):
    """Build the per-core Bass program. Inputs x,y: [b_loc, n, c_in] f32.
    Output "partial": [1,1] f32 = sum of rowmaxes + colmaxes of -pdist."""
    NP = 128                      # partitions
    NG = n // NP                  # point groups per batch
    FD = min(2048, n)             # free-dim per PSUM group (4 banks)
    JG = n // FD                  # j-groups per batch
    NS = FD // 512                # matmuls per group

    nc = bacc.Bacc(
        "TRN2",
        target_bir_lowering=False,
        debug=False,
        enable_asserts=False,
        num_devices=num_devices,
    )

    x_d = nc.declare_dram_parameter("x", [b_loc, n, c_in], F32, isOutput=False).ap()
    y_d = nc.declare_dram_parameter("y", [b_loc, n, c_in], F32, isOutput=False).ap()
    out_d = nc.declare_dram_parameter("partial", [1, 1], F32, isOutput=True).ap()

    with tile.TileContext(nc) as tc, ExitStack() as ctx:
        prep = ctx.enter_context(tc.tile_pool(name="prep", bufs=2))
        singles = ctx.enter_context(tc.tile_pool(name="singles", bufs=1))
        bfpool = ctx.enter_context(tc.tile_pool(name="bfpool", bufs=4))
        psum_pool = ctx.enter_context(tc.tile_pool(name="psum", bufs=2, space="PSUM"))
        dram_pool = ctx.enter_context(tc.tile_pool(name="scratch", bufs=1, space="DRAM"))
        smalls = ctx.enter_context(tc.tile_pool(name="smalls", bufs=2))

        # ---- channel-major augmented tensors, one per (batch, side) ----
        chx = [singles.tile([13, n], BF16, tag=f"chx{b}", name=f"chx{b}") for b in range(b_loc)]
        chy = [singles.tile([13, n], BF16, tag=f"chy{b}", name=f"chy{b}") for b in range(b_loc)]

        # ---- prep: build aug in point-major, round-trip through DRAM ----
        for b in range(b_loc):
            for side in ("x", "y"):
                src = x_d if side == "x" else y_d
                xin = prep.tile([NP, NG, c_in], F32, tag="xin")
                # [n, c] -> [p, g, c]  (point index = g*128 + p)
                nc.sync.dma_start(
                    out=xin,
                    in_=src[b].rearrange("(g p) c -> p g c", p=NP),
                )
                aug = prep.tile([NP, NG, 9], BF16, tag="aug")
                sq = prep.tile([NP, NG, 3], F32, tag="sq")
                rt = prep.tile([NP, NG, 1], F32, tag="rt")
                ch = xin[:, :, 0:3]
                if side == "x":
                    # cols: 0-2 xh | 3-5 xl | 6 -rxh | 7 -rxl | 8 -1
                    nc.scalar.copy(aug[:, :, 0:3], ch)
                    nc.vector.tensor_sub(aug[:, :, 3:6], ch, aug[:, :, 0:3])
                    nc.scalar.square(sq, ch)
                    nc.vector.tensor_reduce(rt, sq, axis=AX.X, op=OP.add)
                    nc.scalar.mul(aug[:, :, 6:7], rt, -1.0)
                    # -rx - (-rxh)  =  (rt * -1) - aug6
                    nc.vector.scalar_tensor_tensor(
                        aug[:, :, 7:8], rt, -1.0, aug[:, :, 6:7],
                        OP.mult, OP.subtract,
                    )
                    nc.vector.memset(aug[:, :, 8:9], -1.0)
                else:
                    # cols: 0-2 Yh | 3-5 Yl | 6 +1 | 7 ryh | 8 ryl ; Y = 2y
                    nc.scalar.mul(aug[:, :, 0:3], ch, 2.0)
                    nc.vector.scalar_tensor_tensor(
                        aug[:, :, 3:6], ch, 2.0, aug[:, :, 0:3],
                        OP.mult, OP.subtract,
                    )
                    nc.vector.memset(aug[:, :, 6:7], 1.0)
                    nc.scalar.square(sq, ch)
                    nc.vector.tensor_reduce(rt, sq, axis=AX.X, op=OP.add)
                    nc.scalar.copy(aug[:, :, 7:8], rt)
                    nc.vector.tensor_sub(aug[:, :, 8:9], rt, aug[:, :, 7:8])

                scr = dram_pool.tile([n, 9], BF16, tag=f"scr_{side}{b}")
                nc.sync.dma_start(
                    out=scr.rearrange("(g p) c -> p g c", p=NP), in_=aug
                )
                # readback: transpose + row duplication via affine APs
                dst = chx[b] if side == "x" else chy[b]

                def col_rows(first_col, ncols, _scr=scr):
                    """src AP iterating [ncols, n] over DRAM scratch [n, 9]:
                    element (pt, col) at offset pt*9 + col."""
                    return bass.AP(
                        tensor=_scr.tensor,
                        offset=_scr.offset + first_col,
                        ap=[[1, ncols], [9, n]],
                    )

                if side == "x":
                    # rows 0-2,3-5 <- cols 0-2 (xh x2); 6-11 <- cols 3..8; 12 <- col 8
                    nc.sync.dma_start(out=dst[0:3, :], in_=col_rows(0, 3))
                    nc.sync.dma_start(out=dst[3:6, :], in_=col_rows(0, 3))
                    nc.sync.dma_start(out=dst[6:12, :], in_=col_rows(3, 6))
                    nc.sync.dma_start(out=dst[12:13, :], in_=col_rows(8, 1))
                else:
                    # rows 0-5 <- cols 0-5; 6-8 <- cols 0-2 (Yh dup);
                    # 9,10 <- col 6 (+1); 11-12 <- cols 7,8
                    nc.sync.dma_start(out=dst[0:6, :], in_=col_rows(0, 6))
                    nc.sync.dma_start(out=dst[6:9, :], in_=col_rows(0, 3))
                    nc.sync.dma_start(out=dst[9:10, :], in_=col_rows(6, 1))
                    nc.sync.dma_start(out=dst[10:11, :], in_=col_rows(6, 1))
                    nc.sync.dma_start(out=dst[11:13, :], in_=col_rows(7, 2))

        # ---- accumulators ----
        colacc = [
            [singles.tile([NP, FD], BF16, tag=f"colacc{b}_{g}", name=f"colacc{b}_{g}") for g in range(JG)]
            for b in range(b_loc)
        ]
        for b in range(b_loc):
            for g in range(JG):
                nc.vector.memset(colacc[b][g], NEG_BIG)
        rowpart = [
            singles.tile([NP, NG, JG], BF16, tag=f"rowpart{b}", name=f"rowpart{b}") for b in range(b_loc)
        ]

        # ---- main loop ----
        for b in range(b_loc):
            for r in range(NG):
                lhsT = chx[b][:, r * NP : (r + 1) * NP]
                for g in range(JG):
                    ps = psum_pool.tile([NP, FD], F32, tag="ps")
                    for s in range(NS):
                        nc.tensor.matmul(
                            ps[:, s * 512 : (s + 1) * 512],
                            lhsT=lhsT,
                            rhs=chy[b][:, g * FD + s * 512 : g * FD + (s + 1) * 512],
                            start=True,
                            stop=True,
                        )
                    bf = bfpool.tile([NP, FD], BF16, tag="bf")
                    nc.scalar.copy(bf, ps)
                    nc.vector.tensor_reduce(
                        rowpart[b][:, r : r + 1, g : g + 1], bf, axis=AX.X, op=OP.max
                    )
                    nc.vector.tensor_tensor(colacc[b][g], colacc[b][g], bf, op=OP.max)

        # ---- finals ----
        partials = singles.tile([1, 8], F32, tag="partials")
        nc.vector.memset(partials, 0.0)
        for b in range(b_loc):
            rmax = smalls.tile([NP, NG], BF16, tag="rmax")
            nc.vector.tensor_reduce(rmax, rowpart[b], axis=AX.X, op=OP.max)
            rsum = smalls.tile([NP, 1], F32, tag="rsum")
            nc.vector.tensor_reduce(rsum, rmax, axis=AX.X, op=OP.add)
            par_r = smalls.tile([NP, 1], F32, tag="par_r")
            nc.gpsimd.partition_all_reduce(
                par_r, rsum, channels=NP, reduce_op=bass_isa.ReduceOp.add
            )
            nc.vector.tensor_copy(partials[:, b : b + 1], par_r[0:1, :])
            for g in range(JG):
                par_c = smalls.tile([NP, FD], F32, tag="par_c")
                nc.gpsimd.partition_all_reduce(
                    par_c, colacc[b][g], channels=NP,
                    reduce_op=bass_isa.ReduceOp.max,
                )
                ctot = smalls.tile([NP, 1], F32, tag="ctot")
                nc.vector.tensor_reduce(ctot, par_c, axis=AX.X, op=OP.add)
                nc.vector.tensor_copy(
                    partials[:, 2 + b * JG + g : 3 + b * JG + g], ctot[0:1, :]
                )
        total = singles.tile([1, 1], F32, tag="total")
        nc.vector.tensor_reduce(total, partials, axis=AX.X, op=OP.add)
        nc.sync.dma_start(out=out_d, in_=total)

    nc.compile()
    return nc


_CACHE = {}


def _get_nc():
    if "nc" not in _CACHE:
        _CACHE["nc"] = build_nc(
            b_loc=B_FULL // N_CORES, n=N_FULL, c_in=C_FULL, num_devices=N_CORES
        )
    return _CACHE["nc"]


def kernel(x: np.ndarray, y: np.ndarray) -> np.ndarray:
    x = np.ascontiguousarray(np.asarray(x, dtype=np.float32))
    y = np.ascontiguousarray(np.asarray(y, dtype=np.float32))
    assert x.shape == (B_FULL, N_FULL, C_FULL), x.shape
    nc = _get_nc()
    bl = B_FULL // N_CORES
    in_maps = [
        {
            "x": np.ascontiguousarray(x[i * bl : (i + 1) * bl]),
            "y": np.ascontiguousarray(y[i * bl : (i + 1) * bl]),
        }
        for i in range(N_CORES)
    ]
    res = run_bass_kernel_spmd(nc, in_maps, list(range(N_CORES)))
    total = sum(float(r["partial"][0, 0]) for r in res.results)
    loss = -total / float(B_FULL * N_FULL)
    return np.float32(loss)


# revision 10
# speedup vs baseline: 4.1197x; 4.1197x over previous
"""ChamferLoss Trainium2 kernel.

Strategy (per core, data-parallel over batch: 16 batches / 8 cores = 2 each):
  pdist[b,i,j] = ||x_i||^2 + ||y_j||^2 - 2 x_i.y_j   (first 3 channels)
  loss = mean_bj(min_i pdist) + mean_bi(min_j pdist)

We compute m = -pdist via a single K=13 bf16 augmented matmul (split-precision
hi/lo trick gives fp32-class accuracy at bf16 PE speed):
  x-side rows: [xh(3), xh(3), xl(3), -rxh, -rxl, -1, -1]
  y-side rows: [Yh(3), Yl(3), Yh(3),  1,    1,  Ryh, Ryl],  Y = 2y, Ry=||y||^2
  sum_k xrow_k * yrow_k = 2x.y - rx - ry = -pdist
min -> max flip: rowmax via tensor_scalar+accum (4x DVE mode), colmax via
tensor_tensor max accumulation (2x DVE mode).  PSUM tiles are cast to bf16 by
the Scalar engine so the DVE runs packed; bf16 relative rounding of pdist
perturbs the final loss by only ~1e-5 (verified numerically).

Channel-major [13, n] operand layout is produced ON-CHIP with PE transposes
(32x [128,13] per tensor) — strided DRAM round-trips measured ~ms-slow on HW
DMA (tiny segments).  Final partition reductions likewise use PE transposes +
free-axis reduces; gpsimd partition_all_reduce only ever sees [128,1].

Host-side: shard batches across 8 cores, run SPMD, sum per-core partials,
negate, divide by B*N.  Point->partition permutations are irrelevant: both
reductions are permutation-invariant.
"""

from contextlib import ExitStack

import numpy as np

import concourse.bass as bass
import concourse.bacc as bacc
import concourse.tile as tile
from concourse import bass_isa, mybir
from concourse.bass_utils import run_bass_kernel_spmd
from concourse.masks import make_identity

F32 = mybir.dt.float32
BF16 = mybir.dt.bfloat16
AX = mybir.AxisListType
OP = mybir.AluOpType

NEG_BIG = -3.0e38  # ~bf16 lowest; colmax accumulator init

B_FULL = 16
N_FULL = 4096
C_FULL = 6
N_CORES = 8


def build_nc(b_loc=2, n=4096, c_in=6, num_devices=8, reps=1,
             skip_prep=False, skip_finals=False, skip_main=False):
    """Per-core program. Inputs x,y: [b_loc, n, c_in] f32; output "partial"
    [1,1] f32 = sum of rowmaxes + colmaxes of -pdist.  reps>1 re-emits the
    computation for marginal-time benchmarking."""
    NP = 128
    NQ = n // NP                  # points per partition (32)
    FD = min(2048, n)             # free-dim per PSUM group (4 banks)
    JG = n // FD                  # j-groups per batch
    NS = FD // 512                # matmuls per group
    TH = FD // NP                 # transposes per psum tile (16)

    nc = bacc.Bacc(
        "TRN2",
        target_bir_lowering=False,
        debug=False,
        enable_asserts=False,
        num_devices=num_devices,
    )

    x_d = nc.declare_dram_parameter("x", [b_loc, n, c_in], F32, isOutput=False).ap()
    y_d = nc.declare_dram_parameter("y", [b_loc, n, c_in], F32, isOutput=False).ap()
    out_d = nc.declare_dram_parameter("partial", [1, 1], F32, isOutput=True).ap()

    with tile.TileContext(nc) as tc, ExitStack() as ctx:
        prep = ctx.enter_context(tc.tile_pool(name="prep", bufs=2))
        singles = ctx.enter_context(tc.tile_pool(name="singles", bufs=1))
        bfpool = ctx.enter_context(tc.tile_pool(name="bfpool", bufs=4))
        psum_pool = ctx.enter_context(tc.tile_pool(name="psum", bufs=2, space="PSUM"))
        smalls = ctx.enter_context(tc.tile_pool(name="smalls", bufs=2))

        ident = singles.tile([NP, NP], BF16, tag="ident", name="ident")
        make_identity(nc, ident)

        def emit_body():
            # ---- channel-major augmented operands, one per (batch, side) ----
            chx = [singles.tile([13, n], BF16, tag=f"chx{b}", name=f"chx{b}")
                   for b in range(b_loc)]
            chy = [singles.tile([13, n], BF16, tag=f"chy{b}", name=f"chy{b}")
                   for b in range(b_loc)]

            # ---- prep: build aug point-major, PE-transpose to channel-major
            for b in range(b_loc):
                for side in ([] if skip_prep else ("x", "y")):
                    src = x_d if side == "x" else y_d
                    xin = prep.tile([NP, NQ, c_in], F32, tag="xin")
                    # contiguous load: point index = p*NQ + q
                    nc.sync.dma_start(
                        out=xin,
                        in_=src[b].rearrange("(p q) c -> p q c", p=NP),
                    )
                    aug = prep.tile([NP, NQ, 13], BF16, tag="aug")
                    sq = prep.tile([NP, NQ, 3], F32, tag="sq")
                    rt = prep.tile([NP, NQ, 1], F32, tag="rt")
                    ch = xin[:, :, 0:3]
                    nc.scalar.square(sq, ch)
                    nc.vector.tensor_reduce(rt, sq, axis=AX.X, op=OP.add)
                    if side == "x":
                        # [xh xh xl | -rxh -rxl | -1 -1]
                        nc.scalar.copy(aug[:, :, 0:3], ch)
                        nc.vector.tensor_copy(aug[:, :, 3:6], aug[:, :, 0:3])
                        nc.vector.tensor_sub(aug[:, :, 6:9], ch, aug[:, :, 0:3])
                        nc.scalar.mul(aug[:, :, 9:10], rt, -1.0)
                        # -rx - (-rxh)
                        nc.vector.scalar_tensor_tensor(
                            aug[:, :, 10:11], rt, -1.0, aug[:, :, 9:10],
                            OP.mult, OP.subtract,
                        )
                        nc.vector.memset(aug[:, :, 11:13], -1.0)
                    else:
                        # [Yh Yl Yh | 1 1 | ryh ryl],  Y = 2y
                        nc.scalar.mul(aug[:, :, 0:3], ch, 2.0)
                        nc.vector.scalar_tensor_tensor(
                            aug[:, :, 3:6], ch, 2.0, aug[:, :, 0:3],
                            OP.mult, OP.subtract,
                        )
                        nc.vector.tensor_copy(aug[:, :, 6:9], aug[:, :, 0:3])
                        nc.vector.memset(aug[:, :, 9:11], 1.0)
                        nc.scalar.copy(aug[:, :, 11:12], rt)
                        nc.vector.tensor_sub(aug[:, :, 12:13], rt, aug[:, :, 11:12])

                    dst = chx[b] if side == "x" else chy[b]
                    for h in range(NQ // TH):
                        pt = psum_pool.tile([NP, FD], BF16, tag="ps")
                        for t in range(TH):
                            q = h * TH + t
                            nc.tensor.transpose(
                                pt[0:13, t * NP : (t + 1) * NP],
                                aug[:, q, :],
                                ident,
                            )
                        nc.scalar.copy(dst[:, h * FD : (h + 1) * FD], pt[0:13, :])

            # ---- accumulators ----
            colacc = [
                [singles.tile([NP, FD], BF16, tag=f"colacc{b}_{g}",
                              name=f"colacc{b}_{g}") for g in range(JG)]
                for b in range(b_loc)
            ]
            for b in range(b_loc):
                for g in range(JG):
                    nc.vector.memset(colacc[b][g], NEG_BIG)
            rowpart = [
                singles.tile([NP, NQ * JG], BF16, tag=f"rowpart{b}",
                             name=f"rowpart{b}") for b in range(b_loc)
            ]
            junk = singles.tile([NP, FD], BF16, tag="junk", name="junk")

            # ---- main loop ----
            for b in range(b_loc):
                for r in range(0 if skip_main else NQ):
                    lhsT = chx[b][:, r * NP : (r + 1) * NP]
                    for g in range(JG):
                        ps = psum_pool.tile([NP, FD], F32, tag="ps")
                        for s in range(NS):
                            nc.tensor.matmul(
                                ps[:, s * 512 : (s + 1) * 512],
                                lhsT=lhsT,
                                rhs=chy[b][:, g * FD + s * 512 : g * FD + (s + 1) * 512],
                                start=True,
                                stop=True,
                            )
                        bf = bfpool.tile([NP, FD], BF16, tag="bf")
                        nc.scalar.copy(bf, ps)
                        # R1 rowmax: out = max(bf,-BIG) = bf (junk), accum =
                        # max-reduce over free (op1) -> 4x DVE mode
                        nc.vector.tensor_scalar(
                            out=junk,
                            in0=bf,
                            scalar1=NEG_BIG,
                            scalar2=None,
                            op0=OP.max,
                            op1=OP.max,
                            accum_out=rowpart[b][:, r * JG + g : r * JG + g + 1],
                        )
                        nc.vector.tensor_tensor(colacc[b][g], colacc[b][g], bf,
                                                op=OP.max)

            # ---- finals ----
            sums = singles.tile([NP, 8], F32, tag="sums", name="sums")
            nc.vector.memset(sums, 0.0)
            for b in range(0 if skip_finals else b_loc):
                rmax = smalls.tile([NP, NQ], BF16, tag="rmax")
                nc.vector.tensor_reduce(
                    rmax, rowpart[b].rearrange("p (r g) -> p r g", g=JG),
                    axis=AX.X, op=OP.max,
                )
                nc.vector.tensor_reduce(sums[:, b : b + 1], rmax, axis=AX.X,
                                        op=OP.add)
                for g in range(JG):
                    pt2 = psum_pool.tile([NP, FD], BF16, tag="ps")
                    for t in range(TH):
                        nc.tensor.transpose(
                            pt2[:, t * NP : (t + 1) * NP],
                            colacc[b][g][:, t * NP : (t + 1) * NP],
                            ident,
                        )
                    cmax = smalls.tile([NP, TH], F32, tag="cmax")  # f32 out of bf16 PSUM reduce
                    nc.vector.tensor_reduce(
                        cmax, pt2.rearrange("p (t v) -> p t v", t=TH),
                        axis=AX.X, op=OP.max,
                    )
                    nc.vector.tensor_reduce(
                        sums[:, 2 + b * JG + g : 3 + b * JG + g], cmax,
                        axis=AX.X, op=OP.add,
                    )
            persum = smalls.tile([NP, 1], F32, tag="persum")
            nc.vector.tensor_reduce(persum, sums, axis=AX.X, op=OP.add)
            par = smalls.tile([NP, 1], F32, tag="par")
            nc.gpsimd.partition_all_reduce(
                par, persum, channels=NP, reduce_op=bass_isa.ReduceOp.add
            )
            nc.sync.dma_start(out=out_d, in_=par[0:1, :])

        for _ in range(reps):
            emit_body()

    nc.compile()
    return nc


_CACHE = {}


def _get_nc():
    if "nc" not in _CACHE:
        _CACHE["nc"] = build_nc(
            b_loc=B_FULL // N_CORES, n=N_FULL, c_in=C_FULL, num_devices=N_CORES
        )
    return _CACHE["nc"]


def kernel(x: np.ndarray, y: np.ndarray) -> np.ndarray:
    x = np.ascontiguousarray(np.asarray(x, dtype=np.float32))
    y = np.ascontiguousarray(np.asarray(y, dtype=np.float32))
    assert x.shape == (B_FULL, N_FULL, C_FULL), x.shape
    nc = _get_nc()
    bl = B_FULL // N_CORES
    in_maps = [
        {
            "x": np.ascontiguousarray(x[i * bl : (i + 1) * bl]),
            "y": np.ascontiguousarray(y[i * bl : (i + 1) * bl]),
        }
        for i in range(N_CORES)
    ]
    res = run_bass_kernel_spmd(nc, in_maps, list(range(N_CORES)))
    total = sum(float(r["partial"][0, 0]) for r in res.results)
    loss = -total / float(B_FULL * N_FULL)
    return np.float32(loss)


# revision 14
# speedup vs baseline: 6.6512x; 1.6145x over previous
"""ChamferLoss Trainium2 kernel.

Strategy (per core, data-parallel over batch: 16 batches / 8 cores = 2 each):
  pdist[b,i,j] = ||x_i||^2 + ||y_j||^2 - 2 x_i.y_j   (first 3 channels)
  loss = mean_bj(min_i pdist) + mean_bi(min_j pdist)

m = -pdist comes from a single K=13 bf16 augmented matmul (hi/lo split gives
fp32-class accuracy at bf16 PE speed):
  x-side rows: [xh(3), xh(3), xl(3), -rxh, -rxl, -1, -1]
  y-side rows: [Yh(3), Yl(3), Yh(3),  1,    1,  Ryh, Ryl],  Y = 2y, Ry=||y||^2
min -> max flip: rowmax via tensor_scalar+accum, colmax via tensor_tensor max.

CRITICAL environment fact (measured): on this axon execution path every
cross-engine semaphore dependency costs ~30-70 us, while back-to-back work on
one engine runs at full speed.  The kernel is therefore structured to minimise
cross-engine edges, not engine-seconds:
  - one full-PSUM fill per (batch,row-tile): 8 matmuls -> [128,4096] f32,
    then exactly one PE->DVE handoff and one DVE->PE handback (64 fills/core)
  - no ACT cast; DVE reduces straight from PSUM in fp32
  - all prep arithmetic on DVE only; PE transposes build the channel-major
    operands on-chip (no strided DRAM round-trips - those are ms-slow)
  - output is per-partition partial sums [128,4]; the host does the final
    128-way gather-sum (pure unsharding arithmetic)
"""

from contextlib import ExitStack

import numpy as np

import concourse.bass as bass
import concourse.bacc as bacc
import concourse.tile as tile
from concourse import bass_isa, mybir
from concourse.bass_utils import run_bass_kernel_spmd
from concourse.masks import make_identity

F32 = mybir.dt.float32
BF16 = mybir.dt.bfloat16
AX = mybir.AxisListType
OP = mybir.AluOpType

NEG_BIG = -3.0e38

B_FULL = 16
N_FULL = 4096
C_FULL = 6
N_CORES = 8


def build_nc(b_loc=2, n=4096, c_in=6, num_devices=8, reps=1):
    """Per-core program. Inputs x,y: [b_loc, n, c_in] f32; output "partial"
    [128, 2*b_loc] f32 per-partition partial sums of rowmax/colmax of -pdist."""
    NP = 128
    NQ = n // NP                  # row-tiles per batch (32)
    TH = NQ                       # transposes per prep psum fill

    nc = bacc.Bacc(
        "TRN2",
        target_bir_lowering=False,
        debug=False,
        enable_asserts=False,
        num_devices=num_devices,
    )

    x_d = nc.declare_dram_parameter("x", [b_loc, n, c_in], F32, isOutput=False).ap()
    y_d = nc.declare_dram_parameter("y", [b_loc, n, c_in], F32, isOutput=False).ap()
    out_d = nc.declare_dram_parameter(
        "partial", [NP, 2 * b_loc], F32, isOutput=True
    ).ap()

    with tile.TileContext(nc) as tc, ExitStack() as ctx:
        prep = ctx.enter_context(tc.tile_pool(name="prep", bufs=2))
        singles = ctx.enter_context(tc.tile_pool(name="singles", bufs=1))
        psum_pool = ctx.enter_context(tc.tile_pool(name="psum", bufs=1, space="PSUM"))
        smalls = ctx.enter_context(tc.tile_pool(name="smalls", bufs=2))

        ident = singles.tile([NP, NP], BF16, tag="ident", name="ident")
        make_identity(nc, ident)
        ident32 = singles.tile([NP, NP], F32, tag="ident32", name="ident32")
        make_identity(nc, ident32)

        def emit_body():
            chx = [singles.tile([13, n], BF16, tag=f"chx{b}", name=f"chx{b}")
                   for b in range(b_loc)]
            chy = [singles.tile([13, n], BF16, tag=f"chy{b}", name=f"chy{b}")
                   for b in range(b_loc)]

            # ---- prep: aug point-major (DVE only), PE-transpose, DVE evac
            for b in range(b_loc):
                for side in ("x", "y"):
                    src = x_d if side == "x" else y_d
                    xin = prep.tile([NP, NQ, c_in], F32, tag="xin")
                    nc.sync.dma_start(
                        out=xin, in_=src[b].rearrange("(p q) c -> p q c", p=NP)
                    )
                    aug = prep.tile([NP, NQ, 13], BF16, tag="aug")
                    sq = prep.tile([NP, NQ, 3], F32, tag="sq")
                    rt = prep.tile([NP, NQ, 1], F32, tag="rt")
                    ch = xin[:, :, 0:3]
                    nc.vector.tensor_mul(sq, ch, ch)
                    nc.vector.tensor_reduce(rt, sq, axis=AX.X, op=OP.add)
                    if side == "x":
                        # [xh xh xl | -rxh -rxl | -1 -1]
                        nc.vector.tensor_copy(aug[:, :, 0:3], ch)
                        nc.vector.tensor_copy(aug[:, :, 3:6], aug[:, :, 0:3])
                        nc.vector.tensor_sub(aug[:, :, 6:9], ch, aug[:, :, 0:3])
                        nc.vector.tensor_scalar_mul(aug[:, :, 9:10], rt, -1.0)
                        nc.vector.scalar_tensor_tensor(
                            aug[:, :, 10:11], rt, -1.0, aug[:, :, 9:10],
                            OP.mult, OP.subtract,
                        )
                        nc.vector.memset(aug[:, :, 11:13], -1.0)
                    else:
                        # [Yh Yl Yh | 1 1 | ryh ryl],  Y = 2y
                        nc.vector.tensor_scalar_mul(aug[:, :, 0:3], ch, 2.0)
                        nc.vector.scalar_tensor_tensor(
                            aug[:, :, 3:6], ch, 2.0, aug[:, :, 0:3],
                            OP.mult, OP.subtract,
                        )
                        nc.vector.tensor_copy(aug[:, :, 6:9], aug[:, :, 0:3])
                        nc.vector.memset(aug[:, :, 9:11], 1.0)
                        nc.vector.tensor_copy(aug[:, :, 11:12], rt)
                        nc.vector.tensor_sub(aug[:, :, 12:13], rt, aug[:, :, 11:12])

                    # one PSUM fill: 32 transposes, then one DVE evacuation
                    pt = psum_pool.tile([NP, n], BF16, tag="ps")
                    for q in range(TH):
                        nc.tensor.transpose(
                            pt[0:13, q * NP : (q + 1) * NP], aug[:, q, :], ident
                        )
                    dst = chx[b] if side == "x" else chy[b]
                    nc.vector.tensor_copy(dst, pt[0:13, :])

            # ---- accumulators (all DVE-resident) ----
            colacc = [singles.tile([NP, n], F32, tag=f"colacc{b}", name=f"colacc{b}")
                      for b in range(b_loc)]
            for b in range(b_loc):
                nc.vector.memset(colacc[b], NEG_BIG)
            rowpart = [singles.tile([NP, NQ], F32, tag=f"rowpart{b}",
                                    name=f"rowpart{b}") for b in range(b_loc)]
            junk = singles.tile([NP, n], F32, tag="junk", name="junk")

            # ---- main: 64 full-PSUM fills, one PE->DVE->PE round-trip each
            for b in range(b_loc):
                for r in range(NQ):
                    lhsT = chx[b][:, r * NP : (r + 1) * NP]
                    ps = psum_pool.tile([NP, n], F32, tag="ps")
                    for s in range(n // 512):
                        nc.tensor.matmul(
                            ps[:, s * 512 : (s + 1) * 512],
                            lhsT=lhsT,
                            rhs=chy[b][:, s * 512 : (s + 1) * 512],
                            start=True,
                            stop=True,
                        )
                    # R1 rowmax: junk write + max-reduce accum over all 4096 j
                    nc.vector.tensor_scalar(
                        out=junk,
                        in0=ps,
                        scalar1=NEG_BIG,
                        scalar2=None,
                        op0=OP.max,
                        op1=OP.max,
                        accum_out=rowpart[b][:, r : r + 1],
                    )
                    # R2 colmax accumulate
                    nc.vector.tensor_tensor(colacc[b], colacc[b], ps, op=OP.max)

            # ---- finals ----
            sums = singles.tile([NP, 2 * b_loc], F32, tag="sums", name="sums")
            for b in range(b_loc):
                # row side: sum of rowmaxes (per partition)
                nc.vector.tensor_reduce(sums[:, b : b + 1], rowpart[b],
                                        axis=AX.X, op=OP.add)
                # col side: transpose colacc, rowmax-reduce, sum
                pt2 = psum_pool.tile([NP, n], F32, tag="ps")
                for t in range(NQ):
                    nc.tensor.transpose(
                        pt2[:, t * NP : (t + 1) * NP],
                        colacc[b][:, t * NP : (t + 1) * NP],
                        ident32,
                    )
                cmax = smalls.tile([NP, NQ], F32, tag="cmax")
                nc.vector.tensor_reduce(
                    cmax, pt2.rearrange("p (t v) -> p t v", t=NQ),
                    axis=AX.X, op=OP.max,
                )
                nc.vector.tensor_reduce(sums[:, b_loc + b : b_loc + b + 1], cmax,
                                        axis=AX.X, op=OP.add)
            nc.sync.dma_start(out=out_d, in_=sums)

        for _ in range(reps):
            emit_body()

    nc.compile()
    return nc


_CACHE = {}


def _get_nc():
    if "nc" not in _CACHE:
        _CACHE["nc"] = build_nc(
            b_loc=B_FULL // N_CORES, n=N_FULL, c_in=C_FULL, num_devices=N_CORES
        )
    return _CACHE["nc"]


def kernel(x: np.ndarray, y: np.ndarray) -> np.ndarray:
    x = np.ascontiguousarray(np.asarray(x, dtype=np.float32))
    y = np.ascontiguousarray(np.asarray(y, dtype=np.float32))
    assert x.shape == (B_FULL, N_FULL, C_FULL), x.shape
    nc = _get_nc()
    bl = B_FULL // N_CORES
    in_maps = [
        {
            "x": np.ascontiguousarray(x[i * bl : (i + 1) * bl]),
            "y": np.ascontiguousarray(y[i * bl : (i + 1) * bl]),
        }
        for i in range(N_CORES)
    ]
    res = run_bass_kernel_spmd(nc, in_maps, list(range(N_CORES)))
    total = sum(float(r["partial"].astype(np.float64).sum()) for r in res.results)
    loss = -total / float(B_FULL * N_FULL)
    return np.float32(loss)
